# revision 12
# baseline (speedup 1.0000x reference)
"""Chamfer loss on 8 Trainium2 NeuronCores.

Data-parallel over batch B=8: one batch element per core. Per core the
[N, M] = [2048, 2048] squared-distance matrix is produced on the
TensorEngine as 4 matmuls per 128-row strip using the expansion
    d2[i,j] = |x_i|^2 + |y_j|^2 - 2 x_i . y_j
with augmented operands (bf16 3-way splits, prepared host-side, O(N)
work). Since sqrt is monotone, row/col minima are taken over d2 and
sqrt is applied to the 2*2048 minima only.

Per strip: ScalarE drains the strip to bf16 (one op per PSUM half);
DVE then does all min work on bf16 SBUF data: a custom fused DVE op
(body=min(Src0,Src1), accum=min — registered via the sanctioned
custom-DVE table mechanism; the stock TENSOR_TENSOR_REDUCE ISA op does
not execute on this runtime) folds the two strip halves and emits the
full 2048-wide row-min in ONE pass, plus the col-min accumulate over
the drained strip at 2x. Column minima across partitions are finished
with 16 PE transposes and one multi-dim reduce. Device ships
per-partition sums of sqrt(min); host finishes with a 128-element sum
per core and the batch mean.
"""

import numpy as np

B, N, M, D = 8, 2048, 2048, 2
P = 128            # partition tile (X rows per strip)
TN = N // P        # 16 strips
NBLK = 512         # matmul moving free dim (one PSUM bank of fp32)
K_AUG = 18         # contraction rows: 6 hi/lo/lolo products per coord + split norms
BIG = 3.0e38

_nc_cache = {}
last_results = None
TRACE = False


def _get_min_reduce_op():
    """Register (idempotently) a fused custom DVE op:
        out = min(in0, in1); accum_out = min(accum_init=s0, min over out)
    i.e. the elementwise fold of the two strip halves plus the full
    row-min as a second output, in a single 1x DVE pass."""
    from concourse import dve_ops
    from concourse.dve_spec import Spec, Src0, Src1, C0, minn, lower
    from concourse.dve_table_gen import dve_ver_for
    from concourse.dve_uop import DveOpSpec

    name = "TT_MIN_RMIN_ANT"
    for op in dve_ops.OPS:
        if op.name == name:
            return op
    my = dve_ops.DveOp(
        name,
        Spec(body=minn(Src0, Src1), accum=minn, accum_init=C0),
        subdim=False,
        uops_sha={},
    )
    row = max(dve_ops._SUB_OPCODE_FOR_NAME.values()) + 1
    assert row < 0x20
    dve_ops.OPS.append(my)
    dve_ops._SUB_OPCODE_FOR_NAME[name] = row
    ver = dve_ver_for("TRN2")
    tmp = DveOpSpec(name=name, opcode=row, uops=lower(my.spec, ver=ver), rd1_en=True)
    object.__setattr__(my, "uops_sha", {ver: tmp.sha(ver)})
    return my


def _build(reps=1):
    """reps>1 wraps the whole computation in a hardware For_i loop —
    used only for steady-state timing measurements."""
    import concourse.bacc as bacc
    import concourse.tile as tile
    from concourse import mybir
    from concourse.masks import make_identity
    from contextlib import nullcontext

    tmr = _get_min_reduce_op()
    f32 = mybir.dt.float32
    bf16 = mybir.dt.bfloat16
    Alu = mybir.AluOpType

    nc = bacc.Bacc(
        "TRN2",
        target_bir_lowering=False,
        debug=False,
        enable_asserts=False,
        num_devices=B,
    )
    lhs_d = nc.dram_tensor("lhs_aug", [K_AUG, N], bf16, kind="ExternalInput")
    rhs_d = nc.dram_tensor("rhs_aug", [K_AUG, M], bf16, kind="ExternalInput")
    out_d = nc.dram_tensor("out", [P, 1], f32, kind="ExternalOutput")

    with tile.TileContext(nc) as tc:
        with (
            tc.tile_pool(name="const", bufs=1) as const,
            tc.tile_pool(name="astrips", bufs=3) as astrips,
            tc.tile_pool(name="psum_d2", bufs=3, space="PSUM") as pd2,
            tc.tile_pool(name="psum_epi", bufs=1, space="PSUM") as pepi,
        ):
            lhsT = const.tile([K_AUG, N], bf16)
            rhsT = const.tile([K_AUG, M], bf16)
            nc.sync.dma_start(out=lhsT, in_=lhs_d.ap())
            nc.scalar.dma_start(out=rhsT, in_=rhs_d.ap())

            ident = const.tile([P, P], bf16)
            make_identity(nc, ident)

            acc = const.tile([P, M], bf16)      # running col-min
            junk = const.tile([P, M // 2], bf16)  # fused-op fold output (unused)
            xy = const.tile([P, 2 * TN], f32)   # [:, :TN] row mins, [:, TN:] col mins
            dist = const.tile([P, 2 * TN], f32)
            sums = const.tile([P, 1], f32)

            # preload the sqrt activation table during the ramp so the
            # ~2.7us ACT_TABLE_LOAD is not paid in the serial tail
            warm = const.tile([1, 1], f32)
            nc.vector.memset(warm, 1.0)
            nc.scalar.sqrt(warm, warm)

            loop_cm = tc.For_i(0, reps, 1) if reps > 1 else nullcontext()
            with loop_cm:
                for s in range(TN):
                    dL = pd2.tile([P, M // 2], f32, name="d2")
                    dR = pd2.tile([P, M // 2], f32, name="d2")
                    for j in range(M // NBLK):
                        dst, o = (dL, j * NBLK) if j < 2 else (dR, (j - 2) * NBLK)
                        nc.tensor.matmul(
                            dst[:, o : o + NBLK],
                            lhsT[:, s * P : (s + 1) * P],
                            rhsT[:, j * NBLK : (j + 1) * NBLK],
                            start=True,
                            stop=True,
                        )
                    # ScalarE drains the strip to bf16 (one op per PSUM
                    # tile; dstrip is one contiguous SBUF tile)
                    dstrip = astrips.tile([P, M], bf16, name="dstrip")
                    nc.scalar.copy(dstrip[:, : M // 2], dL)
                    nc.scalar.copy(dstrip[:, M // 2 :], dR)
                    # fused fold + full row-min in one DVE pass
                    nc.vector._custom_dve(
                        tmr,
                        out=junk,
                        in0=dstrip[:, : M // 2],
                        in1=dstrip[:, M // 2 :],
                        s0=BIG,
                        accum_out=xy[:, s : s + 1],
                    )
                    # col-min accumulate over the drained strip (2x)
                    if s == 0:
                        nc.vector.tensor_copy(acc, dstrip)
                    else:
                        nc.vector.tensor_tensor(acc, acc, dstrip, op=Alu.min)

                # partition-min of acc via PE transposes + one multi-dim reduce
                accT = pepi.tile([P, TN, P], bf16, name="accT")
                for t in range(TN):
                    nc.tensor.transpose(
                        accT[:, t, :], acc[:, t * P : (t + 1) * P], ident
                    )
                nc.vector.tensor_reduce(
                    out=xy[:, TN : 2 * TN],
                    in_=accT,
                    axis=mybir.AxisListType.X,
                    op=Alu.min,
                )
                # d2 minima can round slightly negative; clamp before sqrt
                nc.vector.tensor_scalar_max(xy, xy, 0.0)
                nc.scalar.sqrt(dist, xy)
                nc.vector.reduce_sum(sums, dist, axis=mybir.AxisListType.X)
                nc.sync.dma_start(out=out_d.ap(), in_=sums)

    nc.compile()
    return nc


def _split3(v):
    """3-way bf16 split: v ~= h + l + ll with ~2^-27 relative residual."""
    import ml_dtypes

    bf = ml_dtypes.bfloat16
    h = v.astype(bf)
    r = v - h.astype(np.float32)
    l = r.astype(bf)
    ll = (r - l.astype(np.float32)).astype(bf)
    return h, l, ll


def _prep_core(x, y):
    """Host-side per-core operand prep: O(N) layout, norms, bf16 splits.

    Summing lhsT[k]*rhs[k] over the 18 rows reconstructs
    |x|^2 + |y|^2 - 2 x.y with ~2^-27-scale absolute error (products of
    bf16 values are exact in the fp32 PSUM accumulator; only the
    representation residual and the dropped l*ll cross terms remain).
    Per coordinate (w = -2y): h*h', h*l', l*h', l*l', h*ll', ll*h'.
    Norms enter as 3-way splits against ones.
    """
    import ml_dtypes

    bf = ml_dtypes.bfloat16
    x = np.ascontiguousarray(x, dtype=np.float32)
    y = np.ascontiguousarray(y, dtype=np.float32)
    w = -2.0 * y
    nx = (x.astype(np.float64) ** 2).sum(axis=1).astype(np.float32)
    ny = (y.astype(np.float64) ** 2).sum(axis=1).astype(np.float32)

    lhs = np.empty((K_AUG, N), dtype=bf)
    rhs = np.empty((K_AUG, M), dtype=bf)
    k = 0
    for c in range(2):
        xh, xl, xll = _split3(x[:, c])
        wh, wl, wll = _split3(w[:, c])
        for a, b in ((xh, wh), (xh, wl), (xl, wh), (xl, wl), (xh, wll), (xll, wh)):
            lhs[k], rhs[k] = a, b
            k += 1
    one_n = np.ones(N, bf)
    one_m = np.ones(M, bf)
    for part in _split3(nx):
        lhs[k], rhs[k] = part, one_m
        k += 1
    for part in _split3(ny):
        lhs[k], rhs[k] = one_n, part
        k += 1
    assert k == K_AUG
    return {"lhs_aug": lhs, "rhs_aug": rhs}


def run(pds, pred_pds, reps=1, trace=None):
    global last_results
    from concourse import bass_utils

    pds = np.asarray(pds)
    pred_pds = np.asarray(pred_pds)
    assert pds.shape == (B, N, D) and pred_pds.shape == (B, M, D)

    if reps not in _nc_cache:
        _nc_cache[reps] = _build(reps)
    nc = _nc_cache[reps]

    in_maps = [_prep_core(pds[b], pred_pds[b]) for b in range(B)]
    last_results = bass_utils.run_bass_kernel_spmd(
        nc, in_maps, core_ids=list(range(B)),
        trace=TRACE if trace is None else trace,
    )
    vals = [
        float(last_results.results[b]["out"].sum()) / (2.0 * N) for b in range(B)
    ]
    return np.float32(np.mean(vals))


def kernel(pds, pred_pds):
    return run(pds, pred_pds, reps=1)


# revision 13
# speedup vs baseline: 1.0083x; 1.0083x over previous
"""Chamfer loss on 8 Trainium2 NeuronCores.

Data-parallel over batch B=8: one batch element per core. Per core the
[N, M] = [2048, 2048] squared-distance matrix is produced on the
TensorEngine as 4 matmuls per 128-row strip using the expansion
    d2[i,j] = |x_i|^2 + |y_j|^2 - 2 x_i . y_j
with augmented operands (bf16 3-way splits, prepared host-side, O(N)
work). Since sqrt is monotone, row/col minima are taken over d2 and
sqrt is applied to the 2*2048 minima only.

Per strip: ScalarE drains the strip to bf16 (one op per PSUM half);
DVE then does all min work on bf16 SBUF data: a custom fused DVE op
(body=min(Src0,Src1), accum=min — registered via the sanctioned
custom-DVE table mechanism; the stock TENSOR_TENSOR_REDUCE ISA op does
not execute on this runtime) folds the two strip halves and emits the
full 2048-wide row-min in ONE pass, plus the col-min accumulate over
the drained strip at 2x. Column minima across partitions are finished
with 16 PE transposes and one multi-dim reduce. Device ships
per-partition sums of sqrt(min); host finishes with a 128-element sum
per core and the batch mean.
"""

import numpy as np

B, N, M, D = 8, 2048, 2048, 2
P = 128            # partition tile (X rows per strip)
TN = N // P        # 16 strips
NBLK = 512         # matmul moving free dim (one PSUM bank of fp32)
K_AUG = 18         # contraction rows: 6 hi/lo/lolo products per coord + split norms
BIG = 3.0e38

_nc_cache = {}
last_results = None
TRACE = False


def _get_min_reduce_op():
    """Register (idempotently) a fused custom DVE op:
        out = min(in0, in1); accum_out = min(accum_init=s0, min over out)
    i.e. the elementwise fold of the two strip halves plus the full
    row-min as a second output, in a single 1x DVE pass."""
    from concourse import dve_ops
    from concourse.dve_spec import Spec, Src0, Src1, C0, minn, lower
    from concourse.dve_table_gen import dve_ver_for
    from concourse.dve_uop import DveOpSpec

    name = "TT_MIN_RMIN_ANT"
    for op in dve_ops.OPS:
        if op.name == name:
            return op
    my = dve_ops.DveOp(
        name,
        Spec(body=minn(Src0, Src1), accum=minn, accum_init=C0),
        subdim=False,
        uops_sha={},
    )
    row = max(dve_ops._SUB_OPCODE_FOR_NAME.values()) + 1
    assert row < 0x20
    dve_ops.OPS.append(my)
    dve_ops._SUB_OPCODE_FOR_NAME[name] = row
    ver = dve_ver_for("TRN2")
    tmp = DveOpSpec(name=name, opcode=row, uops=lower(my.spec, ver=ver), rd1_en=True)
    object.__setattr__(my, "uops_sha", {ver: tmp.sha(ver)})
    return my


def _build(reps=1):
    """reps>1 wraps the whole computation in a hardware For_i loop —
    used only for steady-state timing measurements."""
    import concourse.bacc as bacc
    import concourse.tile as tile
    from concourse import mybir
    from concourse.masks import make_identity
    from contextlib import nullcontext

    tmr = _get_min_reduce_op()
    f32 = mybir.dt.float32
    bf16 = mybir.dt.bfloat16
    Alu = mybir.AluOpType

    nc = bacc.Bacc(
        "TRN2",
        target_bir_lowering=False,
        debug=False,
        enable_asserts=False,
        num_devices=B,
    )
    lhs_d = nc.dram_tensor("lhs_aug", [K_AUG, N], bf16, kind="ExternalInput")
    rhs_d = nc.dram_tensor("rhs_aug", [K_AUG, M], bf16, kind="ExternalInput")
    out_d = nc.dram_tensor("out", [P, 1], f32, kind="ExternalOutput")

    with tile.TileContext(nc) as tc:
        with (
            tc.tile_pool(name="const", bufs=1) as const,
            tc.tile_pool(name="astrips", bufs=3) as astrips,
            tc.tile_pool(name="psum_d2", bufs=3, space="PSUM") as pd2,
            tc.tile_pool(name="psum_epi", bufs=1, space="PSUM") as pepi,
        ):
            lhsT = const.tile([K_AUG, N], bf16)
            rhsT = const.tile([K_AUG, M], bf16)
            # chunked loads on two queues: the first strip's operands
            # (lhsT[:, :P] and rhsT[:, :M/2]) land first so matmuls can
            # start while the rest streams in
            nc.sync.dma_start(out=lhsT[:, :P], in_=lhs_d.ap()[:, :P])
            nc.scalar.dma_start(out=rhsT[:, : M // 2], in_=rhs_d.ap()[:, : M // 2])
            nc.scalar.dma_start(out=rhsT[:, M // 2 :], in_=rhs_d.ap()[:, M // 2 :])
            nc.sync.dma_start(out=lhsT[:, P:], in_=lhs_d.ap()[:, P:])

            ident = const.tile([P, P], bf16)
            make_identity(nc, ident)

            acc = const.tile([P, M], bf16)      # running col-min
            junk = const.tile([P, M // 2], bf16)  # fused-op fold output (unused)
            xy = const.tile([P, 2 * TN], f32)   # [:, :TN] row mins, [:, TN:] col mins
            dist = const.tile([P, 2 * TN], f32)
            sums = const.tile([P, 1], f32)

            # preload the sqrt activation table during the ramp so the
            # ~2.7us ACT_TABLE_LOAD is not paid in the serial tail
            warm = const.tile([1, 1], f32)
            nc.vector.memset(warm, 1.0)
            nc.scalar.sqrt(warm, warm)

            loop_cm = tc.For_i(0, reps, 1) if reps > 1 else nullcontext()
            with loop_cm:
                for s in range(TN):
                    dL = pd2.tile([P, M // 2], f32, name="d2")
                    dR = pd2.tile([P, M // 2], f32, name="d2")
                    for j in range(M // NBLK):
                        dst, o = (dL, j * NBLK) if j < 2 else (dR, (j - 2) * NBLK)
                        nc.tensor.matmul(
                            dst[:, o : o + NBLK],
                            lhsT[:, s * P : (s + 1) * P],
                            rhsT[:, j * NBLK : (j + 1) * NBLK],
                            start=True,
                            stop=True,
                        )
                    # ScalarE drains the strip to bf16 (one op per PSUM
                    # tile; dstrip is one contiguous SBUF tile)
                    dstrip = astrips.tile([P, M], bf16, name="dstrip")
                    nc.scalar.copy(dstrip[:, : M // 2], dL)
                    nc.scalar.copy(dstrip[:, M // 2 :], dR)
                    # fused fold + full row-min in one DVE pass
                    nc.vector._custom_dve(
                        tmr,
                        out=junk,
                        in0=dstrip[:, : M // 2],
                        in1=dstrip[:, M // 2 :],
                        s0=BIG,
                        accum_out=xy[:, s : s + 1],
                    )
                    # col-min accumulate over the drained strip (2x); the
                    # last strip is split L/R so the epilogue transposes of
                    # the left half start while the right half accumulates
                    if s == 0:
                        nc.vector.tensor_copy(acc, dstrip)
                    elif s < TN - 1:
                        nc.vector.tensor_tensor(acc, acc, dstrip, op=Alu.min)
                    else:
                        nc.vector.tensor_tensor(
                            acc[:, : M // 2], acc[:, : M // 2],
                            dstrip[:, : M // 2], op=Alu.min,
                        )
                        nc.vector.tensor_tensor(
                            acc[:, M // 2 :], acc[:, M // 2 :],
                            dstrip[:, M // 2 :], op=Alu.min,
                        )

                # partition-min of acc via PE transposes + two multi-dim
                # reduces (first reduce overlaps the second transpose batch)
                accT = pepi.tile([P, TN, P], bf16, name="accT")
                for t in range(TN):
                    nc.tensor.transpose(
                        accT[:, t, :], acc[:, t * P : (t + 1) * P], ident
                    )
                    if t == TN // 2 - 1:
                        nc.vector.tensor_reduce(
                            out=xy[:, TN : TN + TN // 2],
                            in_=accT[:, : TN // 2, :],
                            axis=mybir.AxisListType.X,
                            op=Alu.min,
                        )
                nc.vector.tensor_reduce(
                    out=xy[:, TN + TN // 2 :],
                    in_=accT[:, TN // 2 :, :],
                    axis=mybir.AxisListType.X,
                    op=Alu.min,
                )
                # d2 minima can round slightly negative; clamp before sqrt
                nc.vector.tensor_scalar_max(xy, xy, 0.0)
                nc.scalar.sqrt(dist, xy)
                nc.vector.reduce_sum(sums, dist, axis=mybir.AxisListType.X)
                nc.sync.dma_start(out=out_d.ap(), in_=sums)

    nc.compile()
    return nc


def _split3(v):
    """3-way bf16 split: v ~= h + l + ll with ~2^-27 relative residual."""
    import ml_dtypes

    bf = ml_dtypes.bfloat16
    h = v.astype(bf)
    r = v - h.astype(np.float32)
    l = r.astype(bf)
    ll = (r - l.astype(np.float32)).astype(bf)
    return h, l, ll


def _prep_core(x, y):
    """Host-side per-core operand prep: O(N) layout, norms, bf16 splits.

    Summing lhsT[k]*rhs[k] over the 18 rows reconstructs
    |x|^2 + |y|^2 - 2 x.y with ~2^-27-scale absolute error (products of
    bf16 values are exact in the fp32 PSUM accumulator; only the
    representation residual and the dropped l*ll cross terms remain).
    Per coordinate (w = -2y): h*h', h*l', l*h', l*l', h*ll', ll*h'.
    Norms enter as 3-way splits against ones.
    """
    import ml_dtypes

    bf = ml_dtypes.bfloat16
    x = np.ascontiguousarray(x, dtype=np.float32)
    y = np.ascontiguousarray(y, dtype=np.float32)
    w = -2.0 * y
    nx = (x.astype(np.float64) ** 2).sum(axis=1).astype(np.float32)
    ny = (y.astype(np.float64) ** 2).sum(axis=1).astype(np.float32)

    lhs = np.empty((K_AUG, N), dtype=bf)
    rhs = np.empty((K_AUG, M), dtype=bf)
    k = 0
    for c in range(2):
        xh, xl, xll = _split3(x[:, c])
        wh, wl, wll = _split3(w[:, c])
        for a, b in ((xh, wh), (xh, wl), (xl, wh), (xl, wl), (xh, wll), (xll, wh)):
            lhs[k], rhs[k] = a, b
            k += 1
    one_n = np.ones(N, bf)
    one_m = np.ones(M, bf)
    for part in _split3(nx):
        lhs[k], rhs[k] = part, one_m
        k += 1
    for part in _split3(ny):
        lhs[k], rhs[k] = one_n, part
        k += 1
    assert k == K_AUG
    return {"lhs_aug": lhs, "rhs_aug": rhs}


def run(pds, pred_pds, reps=1, trace=None):
    global last_results
    from concourse import bass_utils

    pds = np.asarray(pds)
    pred_pds = np.asarray(pred_pds)
    assert pds.shape == (B, N, D) and pred_pds.shape == (B, M, D)

    if reps not in _nc_cache:
        _nc_cache[reps] = _build(reps)
    nc = _nc_cache[reps]

    in_maps = [_prep_core(pds[b], pred_pds[b]) for b in range(B)]
    last_results = bass_utils.run_bass_kernel_spmd(
        nc, in_maps, core_ids=list(range(B)),
        trace=TRACE if trace is None else trace,
    )
    vals = [
        float(last_results.results[b]["out"].sum()) / (2.0 * N) for b in range(B)
    ]
    return np.float32(np.mean(vals))


def kernel(pds, pred_pds):
    return run(pds, pred_pds, reps=1)
